# revision 26
# baseline (speedup 1.0000x reference)
# Binary linear: y[b,s,o] = sum_i x[b,s,i] * sign(W)[o,i]
#
# Strategy (8 NeuronCores, data-parallel over tokens):
#   - Host: flatten x to [32768, 768] and shard 8 x [4096, 768]. Per core,
#     pack x as [g=8 groups of 512 tokens][k=6 contraction blocks][128, 512]
#     bf16 slabs (contraction dim on SBUF partitions; every DMA is a fully
#     linear 128KB transfer). Pack sign(W).T bf16 as [os=6 out-slabs]
#     [128 i, 6 k, 128 o] so each out-slab is one linear 196KB transfer.
#   - Device (per core): out[o-block, token] layout - every matmul is
#     lhsT=w[128k,128o], rhs=x[128k,512t] -> psum[128o,512t] (one PSUM bank,
#     N=512 moving). Out-slabs are processed in PAIRS with the k-loop
#     interleaved between the two PSUM banks: a single bank caps the
#     accumulate stream at ~2.0GHz, alternating banks sustains the full
#     ~2.4GHz PE rate. Evictions are DVE/ACT f32->bf16 casts, y stores are
#     linear 128KB DMAs spread over the sync/scalar/gpsimd queues.
#     A short PE warmup covers the DMA launch latency; x rides the sync
#     queue (+ first two slabs on scalar), w rides the vector queue with
#     the first out-slab split per-k so the first matmul starts early.
#     y is stored bf16 (weights are exactly +-1 so only x-bf16 + y-bf16
#     rounding: rel err ~2.4e-3) and upcast on host.
#   - Host: unpack [os][g][128, 512] -> [4, 8192, 768] f32.

import numpy as np

N_CORES = 8
B, S, D_IN, D_OUT = 4, 8192, 768, 768
T_TOTAL = B * S              # 32768 tokens
T_CORE = T_TOTAL // N_CORES  # 4096 tokens per core
P = 128
KB = D_IN // P               # 6 contraction blocks
OS = D_OUT // P              # 6 out-feature slabs
TG = 512                     # tokens per group (one PSUM bank of f32)
G = T_CORE // TG             # 8 groups per core
N_WARMUP = 5

_cache = {}


def _build():
    import concourse.bacc as bacc
    import concourse.mybir as mybir
    import concourse.tile as tile

    f32 = mybir.dt.float32
    bf16 = mybir.dt.bfloat16

    nc = bacc.Bacc(
        "TRN2",
        target_bir_lowering=False,
        debug=False,
        num_devices=N_CORES,
    )

    # p-major x pack: per group the DMA lands [p, k, t] with 6KB contiguous
    # per partition - large rows keep the DMA queues at full rate
    xP = nc.dram_tensor("xP", [G, P, KB, TG], bf16, kind="ExternalInput")
    wP = nc.dram_tensor("wP", [OS, P, KB, P], bf16, kind="ExternalInput")
    yP = nc.dram_tensor("yP", [OS, G, P, TG], bf16, kind="ExternalOutput")

    with tile.TileContext(nc) as tc:
        with (
            tc.tile_pool(name="wpool", bufs=1) as w_pool,
            tc.tile_pool(name="xpool", bufs=1) as x_pool,
            tc.tile_pool(name="ypool", bufs=8) as y_pool,
            tc.tile_pool(name="psum", bufs=6, space="PSUM") as psum_pool,
        ):
            # --- PE warmup: dummy matmuls on zeroed scratch so the PE clock
            # has ramped by the time the first real operands land. ---
            wu = x_pool.tile([P, P + TG], bf16, tag="wu", name="wu", bufs=1)
            nc.gpsimd.memset(wu[:], 0.0)
            wups = psum_pool.tile([P, TG], f32, tag="wups", name="wups", bufs=1)
            for _ in range(N_WARMUP):
                nc.tensor.matmul(
                    wups[:], wu[:, :P], wu[:, P:],
                    start=True, stop=True, skip_group_check=True,
                )
            wu_out = x_pool.tile([P, TG], bf16, tag="wuo", name="wuo", bufs=1)
            nc.vector.tensor_copy(wu_out[:], wups[:])

            # --- head loads interleaved across both HW queues in the order
            # the PE will need them (w: whole 196KB slabs - 1536B partition
            # rows; smaller rows halve the queue's effective rate. x group 0
            # granular per-k 128KB slabs, later groups single 768KB). ---
            wt = [None] * OS

            def w_load(os_, eng):
                t = w_pool.tile([P, KB, P], bf16, tag=f"w{os_}", name=f"w{os_}")
                eng.dma_start(t[:], wP[os_])
                wt[os_] = t

            # group 0 as three k-pair tiles (2KB rows) spread over the two
            # queues in need order; later groups whole (6KB rows)
            x0h = [None] * (KB // 2)

            def x0_load(h, eng):
                t = x_pool.tile([P, 2, TG], bf16, tag=f"x0_{h}", name=f"x0_{h}")
                eng.dma_start(t[:], xP[0, :, 2 * h : 2 * h + 2, :])
                x0h[h] = t

            xg = [None] * G

            def xg_load(g, eng):
                t = x_pool.tile([P, KB, TG], bf16, tag=f"xg{g}", name=f"xg{g}")
                eng.dma_start(t[:], xP[g])
                xg[g] = t

            w_load(0, nc.sync)
            w_load(1, nc.scalar)
            x0_load(0, nc.sync)
            x0_load(1, nc.scalar)
            x0_load(2, nc.sync)
            w_load(2, nc.scalar)
            w_load(3, nc.sync)
            w_load(4, nc.scalar)
            xg_load(1, nc.sync)
            w_load(5, nc.scalar)
            xg_load(2, nc.scalar)
            xg_load(3, nc.sync)
            xg_load(4, nc.scalar)
            xg_load(5, nc.sync)
            xg_load(6, nc.sync)
            xg_load(7, nc.sync)

            def lhsT(os_, k):
                return wt[os_][:, k, :]

            def rhs(g, k):
                return x0h[k // 2][:, k % 2, :] if g == 0 else xg[g][:, k, :]

            # --- main loop: out-slab pairs, k-loop interleaved across the
            # pair's two PSUM banks to sustain the full PE rate ---
            ecnt = 0
            for g in range(G):
                for osp in range(OS // 2):
                    os_a, os_b = 2 * osp, 2 * osp + 1
                    ps_a = psum_pool.tile([P, TG], f32, tag="ps", name=f"ps{g}_{os_a}")
                    ps_b = psum_pool.tile([P, TG], f32, tag="ps", name=f"ps{g}_{os_b}")
                    for k in range(KB):
                        st, sp = k == 0, k == KB - 1
                        nc.tensor.matmul(
                            ps_a[:], lhsT(os_a, k), rhs(g, k), start=st, stop=sp,
                        )
                        nc.tensor.matmul(
                            ps_b[:], lhsT(os_b, k), rhs(g, k), start=st, stop=sp,
                        )
                    for os_, ps in ((os_a, ps_a), (os_b, ps_b)):
                        yt = y_pool.tile([P, TG], bf16, tag="y", name=f"y{g}_{os_}")
                        if g == G - 1 and osp == OS // 2 - 1:
                            # tail pair: halves in parallel on both copy
                            # engines and both DMA queues so the final
                            # receipts land as early as possible
                            h = TG // 2
                            nc.vector.tensor_copy(yt[:, :h], ps[:, :h])
                            nc.scalar.copy(yt[:, h:], ps[:, h:])
                            nc.sync.dma_start(yP[os_, g, :, :h], yt[:, :h])
                            nc.scalar.dma_start(yP[os_, g, :, h:], yt[:, h:])
                            ecnt += 1
                            continue
                        # all evictions on DVE: the scalar engine's DMA
                        # issues would delay them and stall PSUM recycling
                        nc.vector.tensor_copy(yt[:], ps[:])
                        # store queues: scalar while sync still streams x,
                        # alternating afterwards so neither queue backs up
                        # near the tail (gpsimd SWDGE spin-up is too slow)
                        if g <= 4:
                            q = nc.scalar
                        else:
                            q = nc.sync if os_ % 2 == 0 else nc.scalar
                        q.dma_start(yP[os_, g], yt[:])
                        ecnt += 1

    nc.compile()
    return nc


def _get_nc():
    if "nc" not in _cache:
        _cache["nc"] = _build()
    return _cache["nc"]


def _prep_inputs(x, weight):
    import ml_dtypes

    x = np.asarray(x, dtype=np.float32)
    w = np.asarray(weight, dtype=np.float32)
    x2 = x.reshape(N_CORES, T_CORE, D_IN)
    # xPack[c][g, p, k, t] = x2[c, g*TG + t, k*P + p]  (p-major: 6KB rows)
    xPack = np.ascontiguousarray(
        x2.reshape(N_CORES, G, TG, KB, P).transpose(0, 1, 4, 3, 2)
    ).astype(ml_dtypes.bfloat16)
    # wPack[os, p, k, o] = sign(W)[os*P + o, k*P + p]  (+-1/0 exact in bf16)
    S4 = np.sign(w).reshape(OS, P, KB, P)  # [os, o, k, p]
    wPack = np.ascontiguousarray(S4.transpose(0, 3, 2, 1)).astype(ml_dtypes.bfloat16)
    return [{"xP": xPack[c], "wP": wPack} for c in range(N_CORES)]


def _unpack_output(res):
    # yP [OS, G, P(o), TG(t)] -> y_core [T_CORE, D_OUT]
    outs = []
    for r in res.results:
        yp = np.asarray(r["yP"]).astype(np.float32)
        outs.append(yp.transpose(1, 3, 0, 2).reshape(T_CORE, D_OUT))
    return np.concatenate(outs, axis=0).reshape(B, S, D_OUT)


def _install_axon_ntff_hook():
    """The agent image's `antenv` lacks `axon_hooks`; register an equivalent
    module backed by direct ctypes calls into libaxon_pjrt.so so that
    run_bass_kernel_spmd(trace=True) can capture NTFF profiles under axon."""
    import sys

    if "antenv.axon_hooks" in sys.modules:
        return
    import contextlib
    import ctypes
    import types

    so_path = "/opt/axon/libaxon_pjrt.so"
    try:
        lib = ctypes.CDLL(so_path)
    except OSError:
        return
    if not hasattr(lib, "axon_start_nrt_profile"):
        return
    lib.axon_start_nrt_profile.argtypes = [
        ctypes.POINTER(ctypes.c_int64),
        ctypes.c_size_t,
    ]
    lib.axon_start_nrt_profile.restype = ctypes.c_int64
    lib.axon_stop_nrt_profile.argtypes = [ctypes.c_char_p]
    lib.axon_stop_nrt_profile.restype = ctypes.c_int64

    @contextlib.contextmanager
    def _hook(output_dir, device_ids):
        import jax

        jax.devices()
        if device_ids:
            ids = (ctypes.c_int64 * len(device_ids))(*device_ids)
            rc = lib.axon_start_nrt_profile(ids, len(device_ids))
        else:
            rc = lib.axon_start_nrt_profile(None, 0)
        if rc != 0:
            raise RuntimeError(f"axon_start_nrt_profile rc={rc}")
        try:
            yield
        finally:
            n = lib.axon_stop_nrt_profile(str(output_dir).encode())
            print(f"ntff profile: {n} file(s) written to {output_dir}")

    mod = types.ModuleType("antenv.axon_hooks")
    mod.get_axon_ntff_profile_hook = lambda: _hook
    mod.set_axon_ntff_profile_hook = lambda h: None
    sys.modules["antenv.axon_hooks"] = mod


def _run(x, weight, trace=False):
    from concourse.bass_utils import run_bass_kernel_spmd

    if trace:
        _install_axon_ntff_hook()
    nc = _get_nc()
    in_maps = _prep_inputs(x, weight)
    res = run_bass_kernel_spmd(
        nc, in_maps, core_ids=list(range(N_CORES)), trace=trace
    )
    return _unpack_output(res), res


def kernel(x, weight):
    out, _ = _run(x, weight, trace=False)
    return out
